# revision 6
# baseline (speedup 1.0000x reference)
"""Trainium2 Bass kernel v2 for nn_CHILDREN_TENSOR (gnn_message_passing).

Problem: nodes [16, 2048, 128] f32, children [16, 2048, 32] int32.
Output [16, 2048, 32, 128] f32: out[b, n, c, :] = lookup[b, children[b,n,c], :]
where lookup = nodes with row 0 zeroed per batch.

Strategy (data-parallel over batch, 2 per core): keep the whole node
table in SBUF feature-major ([128 feat-partitions x 4096 rows]) and do
the gather ON-CHIP with gpsimd ap_gather (free-dim gather, identical
index stream for all partition groups). Gathered columns are transposed
back to row-major 128x128 tiles on the PE (is_transpose matmul against a
fp32 identity - a pure permutation, bit-exact), drained PSUM->SBUF by
ACT and DVE in 512-column blocks, and stored to HBM as 512 KB
contiguous-per-partition HWDGE writes at copy-block granularity (each
store gated by exactly one copydone; HWDGE is FIFO per issuing engine so
only a group's last block-store carries the buf-recycle semaphore). DMA
then only carries the 2 MB table + 2 MB indices in and the 64 MB result
out - the 64 MB random HBM gather read of the dma_gather design is gone.
A few throwaway identity transposes warm the PE pipeline while the first
gather chunk is in flight so early tiles run at full clock, and the
loads are ordered batch-0-table-first with the index stream split in
pieces so the gather starts ~3 us into the program.
"""

import sys

for _p in ("/opt/trn_rl_repo",):
    if _p not in sys.path:
        sys.path.insert(0, _p)

from contextlib import ExitStack

import numpy as np

import concourse.bacc as bacc
import concourse.mybir as mybir
from concourse.bass_utils import run_bass_kernel_spmd

# Problem constants (hardcoded per harness contract).
B, N, C, F = 16, 2048, 32, 128
N_CORES = 8
B_PER_CORE = B // N_CORES            # 2
ROWS_PER_CORE = B_PER_CORE * N * C   # 131072 output rows per core
TBL_COLS = B_PER_CORE * N            # 4096 table columns (feature-major)

CHUNK = 2048                         # gather columns per ap_gather call
NCHUNK = ROWS_PER_CORE // CHUNK      # 64 per iteration
TPC = CHUNK // 128                   # 16 transpose tiles per chunk
GROUP_ROWS = 8192                    # rows per store
GT = GROUP_ROWS // 128               # 64 tiles per store group
N_GROUPS = ROWS_PER_CORE // GROUP_ROWS               # 16 stores per iteration
BLOCKS = ROWS_PER_CORE // 512        # 256 copy blocks (4 tiles) per iteration
BPG = GT // 4                        # 16 copy blocks per store group

NSEMS = 16
NBUFS = 3                            # store-group SBUF buffers
NPSUM = 6                            # rotating PSUM banks of [128, 512]
GSLOTS = 6                           # gather-chunk pipeline depth


def build_nc(repeat=1, timing_build=False, mode="full",
             do_gather=True, do_pe=True, do_copy=True, do_store=True):
    if mode == "store":
        do_gather = do_pe = do_copy = False
    elif mode == "gather":
        do_pe = do_copy = do_store = False
    elif mode == "nostore":
        do_store = False
    elif mode == "compute":
        do_gather = do_store = False
    nc = bacc.Bacc("TRN2", debug=False, target_bir_lowering=False)

    table = nc.dram_tensor(
        "table", [128, TBL_COLS], mybir.dt.float32,
        kind="Internal" if timing_build else "ExternalInput",
    )
    idxs = nc.dram_tensor(
        "idxs", [128, ROWS_PER_CORE // 16], mybir.dt.int16, kind="ExternalInput"
    )
    ident = nc.dram_tensor("ident", [128, 128], mybir.dt.float32,
                           kind="ExternalInput")
    out = nc.dram_tensor(
        "out", [ROWS_PER_CORE, F], mybir.dt.float32,
        kind="Internal" if timing_build else "ExternalOutput",
    )
    tok = (
        nc.dram_tensor("tok", [1, F], mybir.dt.float32, kind="ExternalOutput")
        if timing_build else None
    )

    with (
        nc.sbuf_tensor("table_sb", [128, TBL_COLS], mybir.dt.float32) as table_sb,
        nc.sbuf_tensor(
            "idx_sb", [128, ROWS_PER_CORE // 16], mybir.dt.int16
        ) as idx_sb,
        nc.sbuf_tensor("ident_sb", [128, 128], mybir.dt.float32) as ident_sb,
        nc.sbuf_tensor("gbuf", [128, GSLOTS, CHUNK], mybir.dt.float32) as gbuf,
        nc.sbuf_tensor("buf", [128, NBUFS, GT, F], mybir.dt.float32) as buf,
        nc.semaphore("load_sem") as load_sem,
        nc.semaphore("qstore_sem") as qstore_sem,
        nc.semaphore("tb0_sem") as tb0_sem,
        nc.semaphore("tb1_sem") as tb1_sem,
        nc.semaphore("idxp0") as idxp0,
        nc.semaphore("idxp1") as idxp1,
        nc.semaphore("idxp2") as idxp2,
        nc.semaphore("idxp3") as idxp3,
        ExitStack() as stack,
        nc.Block() as block,
    ):
        psum = [
            stack.enter_context(
                nc.psum_tensor(f"ps{i}", [128, 512], mybir.dt.float32)
            )
            for i in range(NPSUM)
        ]
        scratch_ps = stack.enter_context(
            nc.psum_tensor("ps_warm", [128, 128], mybir.dt.float32)
        )
        gather_sems = [
            stack.enter_context(nc.semaphore(f"gather_sem{i}"))
            for i in range(NSEMS)
        ]
        blkready_sems = [
            stack.enter_context(nc.semaphore(f"blkready_sem{i}"))
            for i in range(NSEMS)
        ]
        copydone_sems = [
            stack.enter_context(nc.semaphore(f"copydone_sem{i}"))
            for i in range(NSEMS)
        ]
        store_sems = [
            stack.enter_context(nc.semaphore(f"store_sem{i}"))
            for i in range(NSEMS)
        ]

        def rnd(i):
            return i // NSEMS + 1

        IDX_PIECES = 4
        IP_COLS = (ROWS_PER_CORE // 16) // IDX_PIECES
        CHUNKS_PER_PIECE = NCHUNK // IDX_PIECES

        @block.sync
        def _(sync):
            # batch-0 table + ident + idx piece 0 first so the gather and
            # PE pipelines start as early as possible; the rest streams in
            # behind them on dedicated semaphores.
            sync.dma_start(table_sb[:, :N], table[:, :N]).then_inc(tb0_sem, 16)
            sync.dma_start(ident_sb[:], ident[:]).then_inc(load_sem, 16)
            idxp_sems = [idxp0, idxp1, idxp2, idxp3]
            sync.dma_start(idx_sb[:, :IP_COLS],
                           idxs[:, :IP_COLS]).then_inc(idxp0, 16)
            sync.dma_start(table_sb[:, N:], table[:, N:]).then_inc(tb1_sem, 16)
            for ip in range(1, IDX_PIECES):
                sync.dma_start(
                    idx_sb[:, ip * IP_COLS:(ip + 1) * IP_COLS],
                    idxs[:, ip * IP_COLS:(ip + 1) * IP_COLS],
                ).then_inc(idxp_sems[ip], 16)
            # Merged 2-D APs on both sides: per partition one contiguous
            # 32 KB run -> large descriptors.
            out_v = out.rearrange("(s p gf) f -> s p (gf f)", p=128, gf=GT)
            buf_v = buf.rearrange("p n g f -> p n (g f)")
            if do_store:
                # copy-block-granular stores (512 KB): each waits exactly
                # one copydone, so the DMA engines start right behind the
                # first copied block and the drain tail is one block, not
                # a whole 4 MB group. HWDGE is FIFO per issuing engine, so
                # only the last block-store of a group carries the group's
                # buf-recycle semaphore.
                SB = 1
                bsz = SB * 4 * F     # store unit = SB copy blocks
                for gs in range(repeat * N_GROUPS):
                    s = gs % N_GROUPS
                    for k in range(BPG // SB):
                        for q in range(gs * BPG + k * SB,
                                       gs * BPG + (k + 1) * SB):
                            if do_copy:
                                sync.wait_ge(copydone_sems[q % NSEMS], rnd(q))
                        st = sync.dma_start(
                            out_v[s][:, k * bsz:(k + 1) * bsz],
                            buf_v[:, gs % NBUFS, k * bsz:(k + 1) * bsz],
                        )
                        if k == BPG // SB - 1:
                            st.then_inc(store_sems[gs % NSEMS], 16)
                        else:
                            st.then_inc(qstore_sem, 16)
                for i in range(NSEMS):
                    sync.wait_ge(
                        store_sems[i], 16 * (repeat * N_GROUPS // NSEMS)
                    )
            elif do_copy:
                for i in range(NSEMS):
                    sync.wait_ge(
                        copydone_sems[i], repeat * BLOCKS // NSEMS
                    )
            elif do_gather:
                for i in range(NSEMS):
                    sync.wait_ge(
                        gather_sems[i], repeat * NCHUNK // NSEMS
                    )
            if tok is not None:
                sync.dma_start(tok[:], buf[:1, 0, 0, :]).then_inc(load_sem, 16)
                sync.wait_ge(load_sem, 32)

        @block.gpsimd
        def _(gpsimd):
            if not do_gather:
                return
            idxp_sems = [idxp0, idxp1, idxp2, idxp3]
            gpsimd.wait_ge(tb0_sem, 16)
            for gc in range(repeat * NCHUNK):
                c = gc % NCHUNK
                if gc < NCHUNK and c % CHUNKS_PER_PIECE == 0:
                    gpsimd.wait_ge(idxp_sems[c // CHUNKS_PER_PIECE], 16)
                if gc == NCHUNK // B_PER_CORE:
                    gpsimd.wait_ge(tb1_sem, 16)
                bpc = CHUNK // 512
                if gc >= GSLOTS and do_copy:
                    for q in range((gc - GSLOTS) * bpc, (gc - GSLOTS + 1) * bpc):
                        gpsimd.wait_ge(copydone_sems[q % NSEMS], rnd(q))
                # chunks never straddle the batch boundary; use the
                # per-batch 2048-column table slice with batch-local idxs
                b = c // (NCHUNK // B_PER_CORE)
                gpsimd.ap_gather(
                    gbuf[:, gc % GSLOTS],
                    table_sb[:, b * N:(b + 1) * N],
                    idx_sb[:, c * (CHUNK // 16):(c + 1) * (CHUNK // 16)],
                    128,          # channels
                    N,            # num_elems
                    1,            # d
                    CHUNK,        # num_idxs
                ).then_inc(gather_sems[gc % NSEMS], 1)

        @block.tensor
        def _(tensor):
            if not do_pe:
                return
            tensor.wait_ge(load_sem, 16)
            # p-state warmup: keep the PE busy on throwaway identity
            # transposes while the first gather chunk is in flight, so
            # real tiles run at full clock from the start.
            NWARM = 12
            if do_gather:
                for _ in range(NWARM):
                    tensor.matmul(
                        scratch_ps[:], ident_sb[:], ident_sb[:],
                        is_transpose=True, start=True, stop=True,
                    )
            for gk in range(repeat * NCHUNK * TPC):
                gc, t = divmod(gk, TPC)
                q = gk // 4          # global 4-tile copy block
                if t == 0 and do_gather:
                    tensor.wait_ge(gather_sems[gc % NSEMS], rnd(gc))
                if gk % 4 == 0 and q >= NPSUM and do_copy:
                    tensor.wait_ge(copydone_sems[(q - NPSUM) % NSEMS],
                                   rnd(q - NPSUM))
                mm = tensor.matmul(
                    psum[q % NPSUM][:, (gk % 4) * 128:(gk % 4) * 128 + 128],
                    gbuf[:, gc % GSLOTS, t * 128:(t + 1) * 128],
                    ident_sb[:],
                    is_transpose=True,
                    start=True,
                    stop=True,
                )
                mm.then_inc(blkready_sems[q % NSEMS], 1)

        def copy_body(eng, parity):
            if not do_copy:
                return
            for q in range(repeat * BLOCKS):
                if q % 2 != parity:
                    continue
                gq = q // BPG        # global store group
                if do_pe:
                    eng.wait_ge(blkready_sems[q % NSEMS], 4 * rnd(q))
                if gq >= NBUFS and do_store:
                    eng.wait_ge(store_sems[(gq - NBUFS) % NSEMS],
                                16 * rnd(gq - NBUFS))
                qq = q % BPG
                dst = buf[:, gq % NBUFS, qq * 4:(qq + 1) * 4]
                src = psum[q % NPSUM][:]
                cp = (eng.copy(dst, src) if parity == 0
                      else eng.tensor_copy(dst, src))
                cp.then_inc(copydone_sems[q % NSEMS], 1)

        @block.scalar
        def _(scalar):
            copy_body(scalar, 0)

        @block.vector
        def _(vector):
            copy_body(vector, 1)

    nc.compile()
    return nc


def make_in_maps(nodes, children):
    """Host-side shard + layout preprocessing.

    Gather stream position c = s*8192 + t*128 + p produces output row
    s*8192 + p*64 + t (so the PE tile transpose + contiguous store land
    rows in natural order). ap_gather unwraps indices per 16-partition
    group as idx[16g + k%16, base + k//16], identical for all 8 groups.
    """
    nodes_z = np.ascontiguousarray(np.asarray(nodes), dtype=np.float32).copy()
    nodes_z[:, 0, :] = 0.0
    ch = np.asarray(children).astype(np.int64)
    ident = np.ascontiguousarray(np.eye(128, dtype=np.float32))

    in_maps = []
    for core in range(N_CORES):
        nb = nodes_z[core * B_PER_CORE:(core + 1) * B_PER_CORE]
        # feature-major table [128, 4096]
        table = np.ascontiguousarray(
            nb.transpose(2, 0, 1).reshape(F, TBL_COLS).astype(np.float32)
        )
        cb = ch[core * B_PER_CORE:(core + 1) * B_PER_CORE]
        # batch-local indices: each ap_gather call uses its batch's
        # 2048-column table slice
        flat = cb.reshape(ROWS_PER_CORE)
        # row r = s*8192 + p*64 + t  ->  stream position s*8192 + t*128 + p
        a = flat.reshape(N_GROUPS, 128, GT)          # [s, p, t]
        stream = a.transpose(0, 2, 1).reshape(ROWS_PER_CORE)  # [s, t, p]
        # wrap-16: idx16[l, j] = stream[j*16 + l]
        w = stream.reshape(ROWS_PER_CORE // 16, 16).T        # [16, 8192]
        idx16 = np.tile(w, (8, 1)).astype(np.int16)
        in_maps.append(
            {"table": table, "idxs": np.ascontiguousarray(idx16),
             "ident": ident}
        )
    return in_maps


_NC_CACHE = None


def kernel(nodes, children, feature_size=None):
    global _NC_CACHE
    if _NC_CACHE is None:
        _NC_CACHE = build_nc()
    nc = _NC_CACHE

    in_maps = make_in_maps(nodes, children)
    res = run_bass_kernel_spmd(nc, in_maps, list(range(N_CORES))).results

    out = np.empty((B, N, C, F), np.float32)
    for core in range(N_CORES):
        out[core * B_PER_CORE:(core + 1) * B_PER_CORE] = (
            res[core]["out"].reshape(B_PER_CORE, N, C, F)
        )
    return out


# revision 7
# speedup vs baseline: 1.0006x; 1.0006x over previous
"""Trainium2 Bass kernel v2 for nn_CHILDREN_TENSOR (gnn_message_passing).

Problem: nodes [16, 2048, 128] f32, children [16, 2048, 32] int32.
Output [16, 2048, 32, 128] f32: out[b, n, c, :] = lookup[b, children[b,n,c], :]
where lookup = nodes with row 0 zeroed per batch.

Strategy (data-parallel over batch, 2 per core): keep the whole node
table in SBUF feature-major ([128 feat-partitions x 4096 rows]) and do
the gather ON-CHIP with gpsimd ap_gather (free-dim gather, identical
index stream for all partition groups). Gathered columns are transposed
back to row-major 128x128 tiles on the PE (is_transpose matmul against a
fp32 identity - a pure permutation, bit-exact), drained PSUM->SBUF by
ACT and DVE in 512-column blocks, and stored to HBM as 512 KB
contiguous-per-partition HWDGE writes at copy-block granularity (each
store gated by exactly one copydone; HWDGE is FIFO per issuing engine so
only a group's last block-store carries the buf-recycle semaphore). DMA
then only carries the 2 MB table + 2 MB indices in and the 64 MB result
out - the 64 MB random HBM gather read of the dma_gather design is gone.
A few throwaway identity transposes warm the PE pipeline while the first
gather chunk is in flight so early tiles run at full clock, and the
loads are ordered batch-0-table-first with the index stream split in
pieces so the gather starts ~3 us into the program.
"""

import sys

for _p in ("/opt/trn_rl_repo",):
    if _p not in sys.path:
        sys.path.insert(0, _p)

from contextlib import ExitStack

import numpy as np

import concourse.bacc as bacc
import concourse.mybir as mybir
from concourse.bass_utils import run_bass_kernel_spmd

# Problem constants (hardcoded per harness contract).
B, N, C, F = 16, 2048, 32, 128
N_CORES = 8
B_PER_CORE = B // N_CORES            # 2
ROWS_PER_CORE = B_PER_CORE * N * C   # 131072 output rows per core
TBL_COLS = B_PER_CORE * N            # 4096 table columns (feature-major)

CHUNK = 2048                         # gather columns per ap_gather call
NCHUNK = ROWS_PER_CORE // CHUNK      # 64 per iteration
TPC = CHUNK // 128                   # 16 transpose tiles per chunk
GROUP_ROWS = 8192                    # rows per store
GT = GROUP_ROWS // 128               # 64 tiles per store group
N_GROUPS = ROWS_PER_CORE // GROUP_ROWS               # 16 stores per iteration
BLOCKS = ROWS_PER_CORE // 512        # 256 copy blocks (4 tiles) per iteration
BPG = GT // 4                        # 16 copy blocks per store group

NSEMS = 16
NBUFS = 3                            # store-group SBUF buffers
NPSUM = 6                            # rotating PSUM banks of [128, 512]
GSLOTS = 6                           # gather-chunk pipeline depth


def build_nc(repeat=1, timing_build=False, mode="full",
             do_gather=True, do_pe=True, do_copy=True, do_store=True):
    if mode == "store":
        do_gather = do_pe = do_copy = False
    elif mode == "gather":
        do_pe = do_copy = do_store = False
    elif mode == "nostore":
        do_store = False
    elif mode == "compute":
        do_gather = do_store = False
    nc = bacc.Bacc("TRN2", debug=False, target_bir_lowering=False)

    table = nc.dram_tensor(
        "table", [128, TBL_COLS], mybir.dt.float32,
        kind="Internal" if timing_build else "ExternalInput",
    )
    idxs = nc.dram_tensor(
        "idxs", [128, ROWS_PER_CORE // 16], mybir.dt.int16, kind="ExternalInput"
    )
    out = nc.dram_tensor(
        "out", [ROWS_PER_CORE, F], mybir.dt.float32,
        kind="Internal" if timing_build else "ExternalOutput",
    )
    tok = (
        nc.dram_tensor("tok", [1, F], mybir.dt.float32, kind="ExternalOutput")
        if timing_build else None
    )

    with (
        nc.sbuf_tensor("table_sb", [128, TBL_COLS], mybir.dt.float32) as table_sb,
        nc.sbuf_tensor(
            "idx_sb", [128, ROWS_PER_CORE // 16], mybir.dt.int16
        ) as idx_sb,
        nc.sbuf_tensor("ident_sb", [128, 128], mybir.dt.float32) as ident_sb,
        nc.sbuf_tensor("ones_sb", [128, 128], mybir.dt.float32) as ones_sb,
        nc.sbuf_tensor("gbuf", [128, GSLOTS, CHUNK], mybir.dt.float32) as gbuf,
        nc.sbuf_tensor("buf", [128, NBUFS, GT, F], mybir.dt.float32) as buf,
        nc.semaphore("load_sem") as load_sem,
        nc.semaphore("qstore_sem") as qstore_sem,
        nc.semaphore("ident_sem") as ident_sem,
        nc.semaphore("tb0_sem") as tb0_sem,
        nc.semaphore("tb1_sem") as tb1_sem,
        nc.semaphore("idxp0") as idxp0,
        nc.semaphore("idxp1") as idxp1,
        nc.semaphore("idxp2") as idxp2,
        nc.semaphore("idxp3") as idxp3,
        ExitStack() as stack,
        nc.Block() as block,
    ):
        psum = [
            stack.enter_context(
                nc.psum_tensor(f"ps{i}", [128, 512], mybir.dt.float32)
            )
            for i in range(NPSUM)
        ]
        scratch_ps = stack.enter_context(
            nc.psum_tensor("ps_warm", [128, 128], mybir.dt.float32)
        )
        gather_sems = [
            stack.enter_context(nc.semaphore(f"gather_sem{i}"))
            for i in range(NSEMS)
        ]
        blkready_sems = [
            stack.enter_context(nc.semaphore(f"blkready_sem{i}"))
            for i in range(NSEMS)
        ]
        copydone_sems = [
            stack.enter_context(nc.semaphore(f"copydone_sem{i}"))
            for i in range(NSEMS)
        ]
        store_sems = [
            stack.enter_context(nc.semaphore(f"store_sem{i}"))
            for i in range(NSEMS)
        ]

        def rnd(i):
            return i // NSEMS + 1

        IDX_PIECES = 4
        IP_COLS = (ROWS_PER_CORE // 16) // IDX_PIECES
        CHUNKS_PER_PIECE = NCHUNK // IDX_PIECES

        @block.sync
        def _(sync):
            # batch-0 table + ident + idx piece 0 first so the gather and
            # PE pipelines start as early as possible; the rest streams in
            # behind them on dedicated semaphores.
            sync.dma_start(table_sb[:, :N], table[:, :N]).then_inc(tb0_sem, 16)
            idxp_sems = [idxp0, idxp1, idxp2, idxp3]
            sync.dma_start(idx_sb[:, :IP_COLS],
                           idxs[:, :IP_COLS]).then_inc(idxp0, 16)
            sync.dma_start(table_sb[:, N:], table[:, N:]).then_inc(tb1_sem, 16)
            for ip in range(1, IDX_PIECES):
                sync.dma_start(
                    idx_sb[:, ip * IP_COLS:(ip + 1) * IP_COLS],
                    idxs[:, ip * IP_COLS:(ip + 1) * IP_COLS],
                ).then_inc(idxp_sems[ip], 16)
            # Merged 2-D APs on both sides: per partition one contiguous
            # 32 KB run -> large descriptors.
            out_v = out.rearrange("(s p gf) f -> s p (gf f)", p=128, gf=GT)
            buf_v = buf.rearrange("p n g f -> p n (g f)")
            if do_store:
                # copy-block-granular stores (512 KB): each waits exactly
                # one copydone, so the DMA engines start right behind the
                # first copied block and the drain tail is one block, not
                # a whole 4 MB group. HWDGE is FIFO per issuing engine, so
                # only the last block-store of a group carries the group's
                # buf-recycle semaphore.
                SB = 1
                bsz = SB * 4 * F     # store unit = SB copy blocks
                for gs in range(repeat * N_GROUPS):
                    s = gs % N_GROUPS
                    for k in range(BPG // SB):
                        for q in range(gs * BPG + k * SB,
                                       gs * BPG + (k + 1) * SB):
                            if do_copy:
                                sync.wait_ge(copydone_sems[q % NSEMS], rnd(q))
                        st = sync.dma_start(
                            out_v[s][:, k * bsz:(k + 1) * bsz],
                            buf_v[:, gs % NBUFS, k * bsz:(k + 1) * bsz],
                        )
                        if k == BPG // SB - 1:
                            st.then_inc(store_sems[gs % NSEMS], 16)
                        else:
                            st.then_inc(qstore_sem, 16)
                for i in range(NSEMS):
                    sync.wait_ge(
                        store_sems[i], 16 * (repeat * N_GROUPS // NSEMS)
                    )
            elif do_copy:
                for i in range(NSEMS):
                    sync.wait_ge(
                        copydone_sems[i], repeat * BLOCKS // NSEMS
                    )
            elif do_gather:
                for i in range(NSEMS):
                    sync.wait_ge(
                        gather_sems[i], repeat * NCHUNK // NSEMS
                    )
            if tok is not None:
                sync.dma_start(tok[:], buf[:1, 0, 0, :]).then_inc(load_sem, 16)
                sync.wait_ge(load_sem, 16)

        @block.gpsimd
        def _(gpsimd):
            # build the transpose identity on-chip: one less DMA on the
            # serialized DMA-engine budget, and it is ready ~0.3 us in.
            if do_pe:
                gpsimd.memset(ones_sb[:], 1.0).then_inc(ident_sem, 1)
                gpsimd.wait_ge(ident_sem, 1)
                gpsimd.affine_select(
                    ident_sb[:], ones_sb[:], [[-1, 128]],
                    mybir.AluOpType.is_equal, 0.0,
                    base=0, channel_multiplier=1,
                ).then_inc(ident_sem, 1)
            if not do_gather:
                return
            idxp_sems = [idxp0, idxp1, idxp2, idxp3]
            gpsimd.wait_ge(tb0_sem, 16)
            for gc in range(repeat * NCHUNK):
                c = gc % NCHUNK
                if gc < NCHUNK and c % CHUNKS_PER_PIECE == 0:
                    gpsimd.wait_ge(idxp_sems[c // CHUNKS_PER_PIECE], 16)
                if gc == NCHUNK // B_PER_CORE:
                    gpsimd.wait_ge(tb1_sem, 16)
                bpc = CHUNK // 512
                if gc >= GSLOTS and do_copy:
                    for q in range((gc - GSLOTS) * bpc, (gc - GSLOTS + 1) * bpc):
                        gpsimd.wait_ge(copydone_sems[q % NSEMS], rnd(q))
                # chunks never straddle the batch boundary; use the
                # per-batch 2048-column table slice with batch-local idxs
                b = c // (NCHUNK // B_PER_CORE)
                gpsimd.ap_gather(
                    gbuf[:, gc % GSLOTS],
                    table_sb[:, b * N:(b + 1) * N],
                    idx_sb[:, c * (CHUNK // 16):(c + 1) * (CHUNK // 16)],
                    128,          # channels
                    N,            # num_elems
                    1,            # d
                    CHUNK,        # num_idxs
                ).then_inc(gather_sems[gc % NSEMS], 1)

        @block.tensor
        def _(tensor):
            if not do_pe:
                return
            tensor.wait_ge(ident_sem, 2)
            if do_gather:
                tensor.wait_ge(tb0_sem, 16)
            # p-state warmup: keep the PE busy on throwaway identity
            # transposes while the first gather chunk is in flight, so
            # real tiles run at full clock from the start.
            NWARM = 12
            if do_gather:
                for _ in range(NWARM):
                    tensor.matmul(
                        scratch_ps[:], ident_sb[:], ident_sb[:],
                        is_transpose=True, start=True, stop=True,
                    )
            for gk in range(repeat * NCHUNK * TPC):
                gc, t = divmod(gk, TPC)
                q = gk // 4          # global 4-tile copy block
                if t == 0 and do_gather:
                    tensor.wait_ge(gather_sems[gc % NSEMS], rnd(gc))
                if gk % 4 == 0 and q >= NPSUM and do_copy:
                    tensor.wait_ge(copydone_sems[(q - NPSUM) % NSEMS],
                                   rnd(q - NPSUM))
                mm = tensor.matmul(
                    psum[q % NPSUM][:, (gk % 4) * 128:(gk % 4) * 128 + 128],
                    gbuf[:, gc % GSLOTS, t * 128:(t + 1) * 128],
                    ident_sb[:],
                    is_transpose=True,
                    start=True,
                    stop=True,
                )
                mm.then_inc(blkready_sems[q % NSEMS], 1)

        def copy_body(eng, parity):
            if not do_copy:
                return
            for q in range(repeat * BLOCKS):
                if q % 2 != parity:
                    continue
                gq = q // BPG        # global store group
                if do_pe:
                    eng.wait_ge(blkready_sems[q % NSEMS], 4 * rnd(q))
                if gq >= NBUFS and do_store:
                    eng.wait_ge(store_sems[(gq - NBUFS) % NSEMS],
                                16 * rnd(gq - NBUFS))
                qq = q % BPG
                dst = buf[:, gq % NBUFS, qq * 4:(qq + 1) * 4]
                src = psum[q % NPSUM][:]
                cp = (eng.copy(dst, src) if parity == 0
                      else eng.tensor_copy(dst, src))
                cp.then_inc(copydone_sems[q % NSEMS], 1)

        @block.scalar
        def _(scalar):
            copy_body(scalar, 0)

        @block.vector
        def _(vector):
            copy_body(vector, 1)

    nc.compile()
    return nc


def make_in_maps(nodes, children):
    """Host-side shard + layout preprocessing.

    Gather stream position c = s*8192 + t*128 + p produces output row
    s*8192 + p*64 + t (so the PE tile transpose + contiguous store land
    rows in natural order). ap_gather unwraps indices per 16-partition
    group as idx[16g + k%16, base + k//16], identical for all 8 groups.
    """
    nodes_z = np.ascontiguousarray(np.asarray(nodes), dtype=np.float32).copy()
    nodes_z[:, 0, :] = 0.0
    ch = np.asarray(children).astype(np.int64)

    in_maps = []
    for core in range(N_CORES):
        nb = nodes_z[core * B_PER_CORE:(core + 1) * B_PER_CORE]
        # feature-major table [128, 4096]
        table = np.ascontiguousarray(
            nb.transpose(2, 0, 1).reshape(F, TBL_COLS).astype(np.float32)
        )
        cb = ch[core * B_PER_CORE:(core + 1) * B_PER_CORE]
        # batch-local indices: each ap_gather call uses its batch's
        # 2048-column table slice
        flat = cb.reshape(ROWS_PER_CORE)
        # row r = s*8192 + p*64 + t  ->  stream position s*8192 + t*128 + p
        a = flat.reshape(N_GROUPS, 128, GT)          # [s, p, t]
        stream = a.transpose(0, 2, 1).reshape(ROWS_PER_CORE)  # [s, t, p]
        # wrap-16: idx16[l, j] = stream[j*16 + l]
        w = stream.reshape(ROWS_PER_CORE // 16, 16).T        # [16, 8192]
        idx16 = np.tile(w, (8, 1)).astype(np.int16)
        in_maps.append(
            {"table": table, "idxs": np.ascontiguousarray(idx16)}
        )
    return in_maps


_NC_CACHE = None


def kernel(nodes, children, feature_size=None):
    global _NC_CACHE
    if _NC_CACHE is None:
        _NC_CACHE = build_nc()
    nc = _NC_CACHE

    in_maps = make_in_maps(nodes, children)
    res = run_bass_kernel_spmd(nc, in_maps, list(range(N_CORES))).results

    out = np.empty((B, N, C, F), np.float32)
    for core in range(N_CORES):
        out[core * B_PER_CORE:(core + 1) * B_PER_CORE] = (
            res[core]["out"].reshape(B_PER_CORE, N, C, F)
        )
    return out


# revision 8
# speedup vs baseline: 1.0028x; 1.0022x over previous
"""Trainium2 Bass kernel v2 for nn_CHILDREN_TENSOR (gnn_message_passing).

Problem: nodes [16, 2048, 128] f32, children [16, 2048, 32] int32.
Output [16, 2048, 32, 128] f32: out[b, n, c, :] = lookup[b, children[b,n,c], :]
where lookup = nodes with row 0 zeroed per batch.

Strategy (data-parallel over batch, 2 per core): keep the whole node
table in SBUF feature-major ([128 feat-partitions x 4096 rows]) and do
the gather ON-CHIP with gpsimd ap_gather (free-dim gather, identical
index stream for all partition groups). Gathered columns are transposed
back to row-major 128x128 tiles on the PE (is_transpose matmul against a
fp32 identity - a pure permutation, bit-exact), drained PSUM->SBUF by
ACT and DVE in 512-column blocks, and stored to HBM as 512 KB
contiguous-per-partition HWDGE writes at copy-block granularity (each
store gated by exactly one copydone; HWDGE is FIFO per issuing engine so
only a group's last block-store carries the buf-recycle semaphore). DMA
then only carries the 2 MB table + 2 MB indices in and the 64 MB result
out - the 64 MB random HBM gather read of the dma_gather design is gone.
A few throwaway identity transposes warm the PE pipeline while the first
gather chunk is in flight so early tiles run at full clock, and the
loads are ordered batch-0-table-first with the index stream split in
pieces so the gather starts ~3 us into the program.
"""

import sys

for _p in ("/opt/trn_rl_repo",):
    if _p not in sys.path:
        sys.path.insert(0, _p)

from contextlib import ExitStack

import numpy as np

import concourse.bacc as bacc
import concourse.mybir as mybir
from concourse.bass_utils import run_bass_kernel_spmd

# Problem constants (hardcoded per harness contract).
B, N, C, F = 16, 2048, 32, 128
N_CORES = 8
B_PER_CORE = B // N_CORES            # 2
ROWS_PER_CORE = B_PER_CORE * N * C   # 131072 output rows per core
TBL_COLS = B_PER_CORE * N            # 4096 table columns (feature-major)

CHUNK = 2048                         # gather columns per ap_gather call
NCHUNK = ROWS_PER_CORE // CHUNK      # 64 per iteration
TPC = CHUNK // 128                   # 16 transpose tiles per chunk
GROUP_ROWS = 8192                    # rows per store
GT = GROUP_ROWS // 128               # 64 tiles per store group
N_GROUPS = ROWS_PER_CORE // GROUP_ROWS               # 16 stores per iteration
BLOCKS = ROWS_PER_CORE // 512        # 256 copy blocks (4 tiles) per iteration
BPG = GT // 4                        # 16 copy blocks per store group

NSEMS = 16
NBUFS = 3                            # store-group SBUF buffers
NPSUM = 6                            # rotating PSUM banks of [128, 512]
GSLOTS = 6                           # gather-chunk pipeline depth


def build_nc(repeat=1, timing_build=False, mode="full",
             do_gather=True, do_pe=True, do_copy=True, do_store=True):
    if mode == "store":
        do_gather = do_pe = do_copy = False
    elif mode == "gather":
        do_pe = do_copy = do_store = False
    elif mode == "nostore":
        do_store = False
    elif mode == "compute":
        do_gather = do_store = False
    nc = bacc.Bacc("TRN2", debug=False, target_bir_lowering=False)

    table = nc.dram_tensor(
        "table", [128, TBL_COLS], mybir.dt.float32,
        kind="Internal" if timing_build else "ExternalInput",
    )
    idxs = nc.dram_tensor(
        "idxs", [128, ROWS_PER_CORE // 16], mybir.dt.int16, kind="ExternalInput"
    )
    out = nc.dram_tensor(
        "out", [ROWS_PER_CORE, F], mybir.dt.float32,
        kind="Internal" if timing_build else "ExternalOutput",
    )
    tok = (
        nc.dram_tensor("tok", [1, F], mybir.dt.float32, kind="ExternalOutput")
        if timing_build else None
    )

    with (
        nc.sbuf_tensor("table_sb", [128, TBL_COLS], mybir.dt.float32) as table_sb,
        nc.sbuf_tensor(
            "idx_sb", [128, ROWS_PER_CORE // 16], mybir.dt.int16
        ) as idx_sb,
        nc.sbuf_tensor("ident_sb", [128, 128], mybir.dt.float32) as ident_sb,
        nc.sbuf_tensor("ones_sb", [128, 128], mybir.dt.float32) as ones_sb,
        nc.sbuf_tensor("gbuf", [128, GSLOTS, CHUNK], mybir.dt.float32) as gbuf,
        nc.sbuf_tensor("buf", [128, NBUFS, GT, F], mybir.dt.float32) as buf,
        nc.semaphore("load_sem") as load_sem,
        nc.semaphore("qstore_sem") as qstore_sem,
        nc.semaphore("ident_sem") as ident_sem,
        nc.semaphore("tb0_sem") as tb0_sem,
        nc.semaphore("tb1_sem") as tb1_sem,
        ExitStack() as stack,
        nc.Block() as block,
    ):
        psum = [
            stack.enter_context(
                nc.psum_tensor(f"ps{i}", [128, 512], mybir.dt.float32)
            )
            for i in range(NPSUM)
        ]
        scratch_ps = stack.enter_context(
            nc.psum_tensor("ps_warm", [128, 128], mybir.dt.float32)
        )
        gather_sems = [
            stack.enter_context(nc.semaphore(f"gather_sem{i}"))
            for i in range(NSEMS)
        ]
        blkready_sems = [
            stack.enter_context(nc.semaphore(f"blkready_sem{i}"))
            for i in range(NSEMS)
        ]
        copydone_sems = [
            stack.enter_context(nc.semaphore(f"copydone_sem{i}"))
            for i in range(NSEMS)
        ]
        store_sems = [
            stack.enter_context(nc.semaphore(f"store_sem{i}"))
            for i in range(NSEMS)
        ]

        def rnd(i):
            return i // NSEMS + 1

        IDX_PIECES = 8
        IP_COLS = (ROWS_PER_CORE // 16) // IDX_PIECES
        CHUNKS_PER_PIECE = NCHUNK // IDX_PIECES
        idxp_sems = [
            stack.enter_context(nc.semaphore(f"idxp{i}"))
            for i in range(IDX_PIECES)
        ]

        @block.sync
        def _(sync):
            # batch-0 table + ident + idx piece 0 first so the gather and
            # PE pipelines start as early as possible; the rest streams in
            # behind them on dedicated semaphores.
            sync.dma_start(table_sb[:, :N], table[:, :N]).then_inc(tb0_sem, 16)
            sync.dma_start(idx_sb[:, :IP_COLS],
                           idxs[:, :IP_COLS]).then_inc(idxp_sems[0], 16)
            sync.dma_start(table_sb[:, N:], table[:, N:]).then_inc(tb1_sem, 16)
            for ip in range(1, IDX_PIECES):
                sync.dma_start(
                    idx_sb[:, ip * IP_COLS:(ip + 1) * IP_COLS],
                    idxs[:, ip * IP_COLS:(ip + 1) * IP_COLS],
                ).then_inc(idxp_sems[ip], 16)
            # Merged 2-D APs on both sides: per partition one contiguous
            # 32 KB run -> large descriptors.
            out_v = out.rearrange("(s p gf) f -> s p (gf f)", p=128, gf=GT)
            buf_v = buf.rearrange("p n g f -> p n (g f)")
            if do_store:
                # copy-block-granular stores (512 KB): each waits exactly
                # one copydone, so the DMA engines start right behind the
                # first copied block and the drain tail is one block, not
                # a whole 4 MB group. HWDGE is FIFO per issuing engine, so
                # only the last block-store of a group carries the group's
                # buf-recycle semaphore.
                SB = 1
                bsz = SB * 4 * F     # store unit = SB copy blocks
                for gs in range(repeat * N_GROUPS):
                    s = gs % N_GROUPS
                    for k in range(BPG // SB):
                        for q in range(gs * BPG + k * SB,
                                       gs * BPG + (k + 1) * SB):
                            if do_copy:
                                sync.wait_ge(copydone_sems[q % NSEMS], rnd(q))
                        st = sync.dma_start(
                            out_v[s][:, k * bsz:(k + 1) * bsz],
                            buf_v[:, gs % NBUFS, k * bsz:(k + 1) * bsz],
                        )
                        if k == BPG // SB - 1:
                            st.then_inc(store_sems[gs % NSEMS], 16)
                        else:
                            st.then_inc(qstore_sem, 16)
                for i in range(NSEMS):
                    sync.wait_ge(
                        store_sems[i], 16 * (repeat * N_GROUPS // NSEMS)
                    )
            elif do_copy:
                for i in range(NSEMS):
                    sync.wait_ge(
                        copydone_sems[i], repeat * BLOCKS // NSEMS
                    )
            elif do_gather:
                for i in range(NSEMS):
                    sync.wait_ge(
                        gather_sems[i], repeat * NCHUNK // NSEMS
                    )
            if tok is not None:
                sync.dma_start(tok[:], buf[:1, 0, 0, :]).then_inc(load_sem, 16)
                sync.wait_ge(load_sem, 16)

        @block.gpsimd
        def _(gpsimd):
            # build the transpose identity on-chip: one less DMA on the
            # serialized DMA-engine budget, and it is ready ~0.3 us in.
            if do_pe:
                gpsimd.memset(ones_sb[:], 1.0).then_inc(ident_sem, 1)
                gpsimd.wait_ge(ident_sem, 1)
                gpsimd.affine_select(
                    ident_sb[:], ones_sb[:], [[-1, 128]],
                    mybir.AluOpType.is_equal, 0.0,
                    base=0, channel_multiplier=1,
                ).then_inc(ident_sem, 1)
            if not do_gather:
                return
            gpsimd.wait_ge(tb0_sem, 16)
            for gc in range(repeat * NCHUNK):
                c = gc % NCHUNK
                if gc < NCHUNK and c % CHUNKS_PER_PIECE == 0:
                    gpsimd.wait_ge(idxp_sems[c // CHUNKS_PER_PIECE], 16)
                if gc == NCHUNK // B_PER_CORE:
                    gpsimd.wait_ge(tb1_sem, 16)
                bpc = CHUNK // 512
                if gc >= GSLOTS and do_copy:
                    for q in range((gc - GSLOTS) * bpc, (gc - GSLOTS + 1) * bpc):
                        gpsimd.wait_ge(copydone_sems[q % NSEMS], rnd(q))
                # chunks never straddle the batch boundary; use the
                # per-batch 2048-column table slice with batch-local idxs
                b = c // (NCHUNK // B_PER_CORE)
                gpsimd.ap_gather(
                    gbuf[:, gc % GSLOTS],
                    table_sb[:, b * N:(b + 1) * N],
                    idx_sb[:, c * (CHUNK // 16):(c + 1) * (CHUNK // 16)],
                    128,          # channels
                    N,            # num_elems
                    1,            # d
                    CHUNK,        # num_idxs
                ).then_inc(gather_sems[gc % NSEMS], 1)

        @block.tensor
        def _(tensor):
            if not do_pe:
                return
            tensor.wait_ge(ident_sem, 2)
            if do_gather:
                tensor.wait_ge(tb0_sem, 16)
            # p-state warmup: keep the PE busy on throwaway identity
            # transposes while the first gather chunk is in flight, so
            # real tiles run at full clock from the start.
            NWARM = 12
            if do_gather:
                for _ in range(NWARM):
                    tensor.matmul(
                        scratch_ps[:], ident_sb[:], ident_sb[:],
                        is_transpose=True, start=True, stop=True,
                    )
            for gk in range(repeat * NCHUNK * TPC):
                gc, t = divmod(gk, TPC)
                q = gk // 4          # global 4-tile copy block
                if t == 0 and do_gather:
                    tensor.wait_ge(gather_sems[gc % NSEMS], rnd(gc))
                if gk % 4 == 0 and q >= NPSUM and do_copy:
                    tensor.wait_ge(copydone_sems[(q - NPSUM) % NSEMS],
                                   rnd(q - NPSUM))
                mm = tensor.matmul(
                    psum[q % NPSUM][:, (gk % 4) * 128:(gk % 4) * 128 + 128],
                    gbuf[:, gc % GSLOTS, t * 128:(t + 1) * 128],
                    ident_sb[:],
                    is_transpose=True,
                    start=True,
                    stop=True,
                )
                mm.then_inc(blkready_sems[q % NSEMS], 1)

        def copy_body(eng, parity):
            if not do_copy:
                return
            for q in range(repeat * BLOCKS):
                if q % 2 != parity:
                    continue
                gq = q // BPG        # global store group
                if do_pe:
                    eng.wait_ge(blkready_sems[q % NSEMS], 4 * rnd(q))
                if gq >= NBUFS and do_store:
                    eng.wait_ge(store_sems[(gq - NBUFS) % NSEMS],
                                16 * rnd(gq - NBUFS))
                qq = q % BPG
                dst = buf[:, gq % NBUFS, qq * 4:(qq + 1) * 4]
                src = psum[q % NPSUM][:]
                cp = (eng.copy(dst, src) if parity == 0
                      else eng.tensor_copy(dst, src))
                cp.then_inc(copydone_sems[q % NSEMS], 1)

        @block.scalar
        def _(scalar):
            copy_body(scalar, 0)

        @block.vector
        def _(vector):
            copy_body(vector, 1)

    nc.compile()
    return nc


def make_in_maps(nodes, children):
    """Host-side shard + layout preprocessing.

    Gather stream position c = s*8192 + t*128 + p produces output row
    s*8192 + p*64 + t (so the PE tile transpose + contiguous store land
    rows in natural order). ap_gather unwraps indices per 16-partition
    group as idx[16g + k%16, base + k//16], identical for all 8 groups.
    """
    nodes_z = np.ascontiguousarray(np.asarray(nodes), dtype=np.float32).copy()
    nodes_z[:, 0, :] = 0.0
    ch = np.asarray(children).astype(np.int64)

    in_maps = []
    for core in range(N_CORES):
        nb = nodes_z[core * B_PER_CORE:(core + 1) * B_PER_CORE]
        # feature-major table [128, 4096]
        table = np.ascontiguousarray(
            nb.transpose(2, 0, 1).reshape(F, TBL_COLS).astype(np.float32)
        )
        cb = ch[core * B_PER_CORE:(core + 1) * B_PER_CORE]
        # batch-local indices: each ap_gather call uses its batch's
        # 2048-column table slice
        flat = cb.reshape(ROWS_PER_CORE)
        # row r = s*8192 + p*64 + t  ->  stream position s*8192 + t*128 + p
        a = flat.reshape(N_GROUPS, 128, GT)          # [s, p, t]
        stream = a.transpose(0, 2, 1).reshape(ROWS_PER_CORE)  # [s, t, p]
        # wrap-16: idx16[l, j] = stream[j*16 + l]
        w = stream.reshape(ROWS_PER_CORE // 16, 16).T        # [16, 8192]
        idx16 = np.tile(w, (8, 1)).astype(np.int16)
        in_maps.append(
            {"table": table, "idxs": np.ascontiguousarray(idx16)}
        )
    return in_maps


_NC_CACHE = None


def kernel(nodes, children, feature_size=None):
    global _NC_CACHE
    if _NC_CACHE is None:
        _NC_CACHE = build_nc()
    nc = _NC_CACHE

    in_maps = make_in_maps(nodes, children)
    res = run_bass_kernel_spmd(nc, in_maps, list(range(N_CORES))).results

    out = np.empty((B, N, C, F), np.float32)
    for core in range(N_CORES):
        out[core * B_PER_CORE:(core + 1) * B_PER_CORE] = (
            res[core]["out"].reshape(B_PER_CORE, N, C, F)
        )
    return out
